# revision 15
# baseline (speedup 1.0000x reference)
"""Benes butterfly network (12 layers, N=4096) on 8 Trainium2 NeuronCores.

Self-contained: takes full inputs, shards batch across 8 cores, runs a
Bass/Tile kernel per core, gathers the full output.

Math: reference layer k is a butterfly with span 2^k:
    h[:, j] <- A_k[j] * h[:, j] + B_k[j] * h[:, j ^ 2^k]
(A_k/B_k extracted from the sparse COO (values, idx_in, idx_out)).

V3 design (prior version spent ~28us of PE on diagonal matmuls whose
only purpose was transposing the output back to [batch, col]; since the
input is already transposed on the host, the output transpose moves to
the host gather too, and the device works column-major end to end):
  - Layers 0..8 composed on the host into dense 128x128 block matrices
    (mst, bf16) with diag(A9*A10*A11) folded in, so the later stt chain
    needs no trailing per-partition rescale:
      p1[t] = sum_{ji<4} M[t, t^ji] @ H0[t^ji]    (PE, N=512, fp32 psum)
  - Layers 9/10/11 as three scalar_tensor_tensor passes over bf16 SBUF
    tiles (per-partition scalar = per-column coefficient ratio):
      Y[t]   = p1[t] + s9[t]  * p1[t^4]    ( = A10*A11*H9[t] )
      Z[t]   = Y[t]  + s10[t] * Y[t^8]     ( = A11*H10[t] )
      H11[t] = Z[t]  + r11[t] * Z[t^16]    ( = H11[t] exactly )
    stt9/stt11 on DVE (bf16 2x mode), stt10 on GPSIMD.
  - Phase-1 psums evacuated to bf16 by ACT in fused [128,1024] copies.
  - Output written column-major (outT [N, BSH] bf16, contiguous 1KB
    lines); the host gather transposes + upcasts.
  - Jobs (residue, m-pair) flat-ordered to interleave residues 0/1
    during the xT load ramp; DMA issues paced by trigger-copies on the
    ACT queue; warmup matmuls lift the PE HAM throttle during the ramp.
"""
import os
import numpy as np
import ml_dtypes

N = 4096
BATCH = 4096
NLAYERS = 12
NCORES = 8
BSH = BATCH // NCORES      # 512 batch rows per core
T = N // 128               # 32 column tiles

_PROGRAM_CACHE = {}
LAST_EXEC_NS = None


def _extract_ab(values, idx_in, idx_out):
    """Per-layer butterfly coefficients A[k], B[k] (float64 [L, N])."""
    v = np.asarray(values, np.float64)
    ii = np.asarray(idx_in, np.int64)
    io = np.asarray(idx_out, np.int64)
    L, nnz = v.shape
    n = nnz // 2
    A = np.zeros((L, n))
    B = np.zeros((L, n))
    for k in range(L):
        s = 1 << k
        self_m = ii[k] == io[k]
        part_m = ii[k] == (io[k] ^ s)
        if not np.all(self_m | part_m):
            raise ValueError(f"layer {k}: unexpected sparse index structure")
        np.add.at(A[k], io[k][self_m], v[k][self_m])
        np.add.at(B[k], io[k][part_m], v[k][part_m])
    return A, B


def _clamp(a):
    return np.where(np.abs(a) < 1e-12, 1e-12, a)


def _host_precompute(values, idx_in, idx_out):
    A, B = _extract_ab(values, idx_in, idx_out)
    Ab = A.reshape(NLAYERS, T, 128)
    Bb = B.reshape(NLAYERS, T, 128)
    j = np.arange(128)

    # Dense composition of within-block layers 0..6, one 128x128 per tile.
    S = [np.eye(128) for _ in range(T)]
    for k in range(7):
        s = 1 << k
        for t in range(T):
            W = np.zeros((128, 128))
            W[j, j] = Ab[k, t]
            W[j, j ^ s] = Bb[k, t]
            S[t] = W @ S[t]
    # Cross-block layers 7, 8 (tile distances 1, 2): dict src_tile -> 128x128
    Sd = [{t: S[t]} for t in range(T)]
    for k in (7, 8):
        d = 1 << (k - 7)
        newS = []
        for t in range(T):
            out = {}
            for src, M in Sd[t].items():
                out[src] = Ab[k, t][:, None] * M
            for src, M in Sd[t ^ d].items():
                out[src] = out.get(src, 0) + Bb[k, t][:, None] * M
            newS.append(out)
        Sd = newS

    D = Ab[9] * Ab[10] * Ab[11]          # folded diagonal, [T, 128]

    # mst (bf16): column block for (r, m) at (r*8 + m)*512, 4 ji-blocks of
    # (diag(D[t]) @ Sd[t][t^ji]).T so matmul computes M @ H0.
    mst = np.zeros((128, T * 512), np.float32)
    for r in range(4):
        for m in range(8):
            t = 4 * m + r
            assert set(Sd[t].keys()) == {t, t ^ 1, t ^ 2, t ^ 3}
            base = (r * 8 + m) * 512
            for ji in range(4):
                M = D[t][:, None] * Sd[t][t ^ ji]
                mst[:, base + ji * 128: base + (ji + 1) * 128] = (
                    M.T.astype(np.float32)
                )

    # combine scalar table, fp32 [128, 96] (AP-scalar ops require fp32):
    # col t = s9, 32+t = s10, 64+t = r11
    stabf = np.zeros((128, 96), np.float64)
    for t in range(T):
        stabf[:, t] = Ab[10, t] * Ab[11, t] * Bb[9, t] / _clamp(D[t ^ 4])
        stabf[:, 32 + t] = (
            Ab[11, t] * Bb[10, t] / _clamp(Ab[10, t ^ 8] * Ab[11, t ^ 8])
        )
        stabf[:, 64 + t] = Bb[11, t] / _clamp(Ab[11, t ^ 16])
    return mst.astype(ml_dtypes.bfloat16), stabf.astype(np.float32)


# Flat job order: (residue, m-pair); residues 0/1 interleaved so the PE
# has runnable work while the tail of xT is still loading.
_JOBS = [
    (0, 0), (0, 1), (1, 0), (0, 2), (1, 1), (0, 3), (1, 2), (1, 3),
    (2, 0), (2, 1), (2, 2), (2, 3), (3, 0), (3, 1), (3, 2), (3, 3),
]


def _build_program():
    import concourse.mybir as mybir
    import concourse.tile as tile
    from concourse import bacc

    f32 = mybir.dt.float32
    bf16 = mybir.dt.bfloat16
    mult = mybir.AluOpType.mult
    add = mybir.AluOpType.add

    nc = bacc.Bacc("TRN2", target_bir_lowering=False, debug=False)
    # xT pre-shuffled on host to each H0 tile's exact SBUF layout: row
    # m*128+p holds [lt, b] contiguously -> 4KB descriptors, 4x fewer.
    xT_ap = nc.dram_tensor("xT", [8 * 128, 2048], bf16, kind="ExternalInput").ap()
    mst_ap = nc.dram_tensor("mst", [128, T * 512], bf16, kind="ExternalInput").ap()
    stabf_ap = nc.dram_tensor("stabf", [128, 96], f32, kind="ExternalInput").ap()
    out_ap = nc.dram_tensor("outT", [N, BSH], bf16, kind="ExternalOutput").ap()

    with tile.TileContext(nc) as tc:
        with (
            tc.tile_pool(name="const", bufs=1) as constp,
            tc.tile_pool(name="h0", bufs=8) as h0p,
            tc.tile_pool(name="mstp", bufs=8) as mstp,
            tc.tile_pool(name="e2", bufs=3) as e2p,
            tc.tile_pool(name="y", bufs=16) as yp,
            tc.tile_pool(name="z", bufs=24) as zp,
            tc.tile_pool(name="h11", bufs=14) as h11p,
            tc.tile_pool(name="trg", bufs=2) as trgp,
            tc.tile_pool(name="wk", bufs=10) as wkp,
            tc.tile_pool(name="ps", bufs=3, space="PSUM") as psp,
            tc.tile_pool(name="pw", bufs=1, space="PSUM") as pwp,
        ):
            stabf = constp.tile([128, 96], f32, name="stabf")

            msth = {}

            def issue_mst(r, half, eng):
                tl = mstp.tile(
                    [128, 2048], bf16, tag="mst", name=f"mst_{r}{half}"
                )
                eng.dma_start(
                    tl[:],
                    mst_ap[:, r * 4096 + half * 2048:
                           r * 4096 + (half + 1) * 2048],
                )
                msth[(r, half)] = tl

            def mst_slice(r, m, ji):
                tl = msth[(r, m // 4)]
                mm = m % 4
                return tl[:, mm * 512 + ji * 128: mm * 512 + (ji + 1) * 128]

            H0 = {}

            def issue_h0(m, eng):
                tl = h0p.tile([128, 2048], bf16, tag="h0", name=f"h0_{m}")
                eng.dma_start(tl[:], xT_ap[m * 128:(m + 1) * 128, :])
                H0[m] = tl

            # upfront, in need-order on one HWDGE ring (FIFO per ring):
            # tiny stabf first (warmup operand), then job 0's inputs
            nc.sync.dma_start(stabf[:], stabf_ap[:])
            issue_mst(0, 0, nc.sync)
            issue_h0(0, nc.sync)
            issue_h0(1, nc.sync)

            # warmup matmuls (need only stabf): hold the PE busy through
            # the load ramp so HAM reaches K=8/8 before real matmuls
            warm = pwp.tile([64, 96], f32, tag="warm", name="warm")
            for _ in range(28):
                nc.tensor.matmul(
                    warm[:], stabf[:, 0:64], stabf[:, 0:96],
                    start=True, stop=True,
                )

            # DMA pacing: trig copies on the ACT queue gate the next issues
            # on an earlier chunk's ARRIVAL (not on compute progress).
            def trig_then(gate_tile, thunks):
                trg = trgp.tile([128, 8], bf16, tag="trg", name="trg")
                nc.scalar.copy(trg[:], gate_tile[:, 0:8])
                for th in thunks:
                    th()

            trig_then(H0[0], [
                lambda: issue_h0(2, nc.scalar),
                lambda: issue_h0(3, nc.scalar),
                lambda: issue_mst(1, 0, nc.scalar),
            ])

            # issues attached after job i's evac (scalar queue, in order)
            sched = {
                0: [lambda: issue_h0(4, nc.scalar),
                    lambda: issue_h0(5, nc.scalar),
                    lambda: issue_mst(0, 1, nc.scalar)],
                1: [lambda: issue_h0(6, nc.scalar),
                    lambda: issue_h0(7, nc.scalar),
                    lambda: issue_mst(1, 1, nc.scalar)],
                2: [lambda: issue_mst(2, 0, nc.scalar)],
                3: [lambda: issue_mst(2, 1, nc.scalar)],
                4: [lambda: issue_mst(3, 0, nc.scalar)],
                5: [lambda: issue_mst(3, 1, nc.scalar)],
            }

            E2 = {}      # (r, mp) -> [128, 1024] bf16 (tiles 4*(2mp)+r | 4*(2mp+1)+r)
            Y = {}
            Z = {}
            H11 = {}
            pend11 = []  # deferred stt11 work items (r, m)

            def Ecol(r, m):
                tl = E2[(r, m >> 1)]
                h = m & 1
                return tl[:, h * 512:(h + 1) * 512]

            def emit_l11(r, m):
                t = 4 * m + r
                H11[t] = h11p.tile([128, BSH], bf16, tag="h11", name=f"H11_{t}")
                nc.vector.scalar_tensor_tensor(
                    H11[t][:], Z[4 * (m ^ 4) + r][:],
                    stabf[:, 64 + t:64 + t + 1], Z[t][:],
                    op0=mult, op1=add,
                )
                nc.sync.dma_start(out_ap[t * 128:(t + 1) * 128, :], H11[t][:])

            for idx, (r, mp) in enumerate(_JOBS):
                # ---- phase 1: 8 matmuls into a [128, 1024] psum pair
                P2 = psp.tile([128, 1024], f32, tag="ps", name=f"p2_{r}_{mp}")
                for h, m in enumerate((2 * mp, 2 * mp + 1)):
                    for ji in range(4):
                        nc.tensor.matmul(
                            P2[:, h * 512:(h + 1) * 512],
                            mst_slice(r, m, ji),
                            H0[m][:, (r ^ ji) * 512:((r ^ ji) + 1) * 512],
                            start=(ji == 0), stop=(ji == 3),
                        )
                # ---- fused ACT evacuation to bf16
                e2 = e2p.tile([128, 1024], bf16, tag="e2", name=f"e2_{r}_{mp}")
                nc.scalar.copy(e2[:], P2[:])
                E2[(r, mp)] = e2
                for th in sched.get(idx, []):
                    th()

                # ---- L9 as fused stt on DVE (decomposed mul->add chains
                # inflate DVE slice times ~30% from pipe-drain stalls)
                for h, m in enumerate((2 * mp, 2 * mp + 1)):
                    t = 4 * m + r
                    Y[t] = yp.tile([128, BSH], bf16, tag="y", name=f"Y_{t}")
                    nc.vector.scalar_tensor_tensor(
                        Y[t][:], Ecol(r, m ^ 1), stabf[:, t:t + 1],
                        Ecol(r, m), op0=mult, op1=add,
                    )

                # ---- drain deferred L11 steps (DVE) + their stores
                for _ in range(3 if idx >= 10 else 2):
                    if pend11:
                        pr, pm = pend11.pop(0)
                        emit_l11(pr, pm)

                # ---- stt10 on GPSIMD per completed half (partners m, m^2)
                if mp in (1, 3):
                    ms = range(0, 4) if mp == 1 else range(4, 8)
                    if r == 3:
                        for m in ms:
                            t = 4 * m + r
                            Z[t] = zp.tile(
                                [128, BSH], bf16, tag="z", name=f"Z_{t}"
                            )
                            nc.vector.scalar_tensor_tensor(
                                Z[t][:], Y[4 * (m ^ 2) + r][:],
                                stabf[:, 32 + t:32 + t + 1], Y[t][:],
                                op0=mult, op1=add,
                            )
                    else:
                        w10 = {}
                        for m in ms:
                            t = 4 * m + r
                            w10[m] = wkp.tile(
                                [128, BSH], bf16, tag="wk", name=f"W10_{t}"
                            )
                            nc.scalar.mul(
                                w10[m][:], Y[4 * (m ^ 2) + r][:],
                                stabf[:, 32 + t:32 + t + 1],
                            )
                        for m in ms:
                            t = 4 * m + r
                            Z[t] = zp.tile(
                                [128, BSH], bf16, tag="z", name=f"Z_{t}"
                            )
                            eng = nc.gpsimd if r < 2 else nc.vector
                            eng.tensor_add(Z[t][:], w10[m][:], Y[t][:])
                if mp == 3:
                    pend11.extend((r, m) for m in range(8))

            # tail: drain remaining L11 steps on DVE
            for pr, pm in pend11:
                emit_l11(pr, pm)

    nc.compile()
    return nc


def kernel(x, values, idx_in, idx_out):
    global LAST_EXEC_NS
    from concourse.bass_utils import run_bass_kernel_spmd

    x = np.asarray(x, np.float32)
    assert x.shape == (BATCH, N), x.shape
    mst, stabf = _host_precompute(values, idx_in, idx_out)
    xT = np.ascontiguousarray(x.T.astype(ml_dtypes.bfloat16))

    if "prog" not in _PROGRAM_CACHE:
        _PROGRAM_CACHE["prog"] = _build_program()
    nc = _PROGRAM_CACHE["prog"]

    in_maps = []
    for i in range(NCORES):
        xc = xT[:, i * BSH:(i + 1) * BSH]                  # [N, 512]
        # row m*128+p holds (lt, b) contiguously = H0 tile SBUF layout
        xs = np.ascontiguousarray(
            xc.reshape(8, 4, 128, BSH).transpose(0, 2, 1, 3).reshape(1024, 2048)
        )
        in_maps.append({"xT": xs, "mst": mst, "stabf": stabf})
    res = run_bass_kernel_spmd(nc, in_maps, core_ids=list(range(NCORES)))
    if os.environ.get("BENES_TRACE"):
        tres = run_bass_kernel_spmd(
            nc, in_maps, core_ids=list(range(NCORES)), trace=True
        )
        LAST_EXEC_NS = tres.exec_time_ns
        _PROGRAM_CACHE["profile_json"] = tres.profile_json
    out = np.empty((BATCH, N), np.float32)
    for i in range(NCORES):
        out[i * BSH:(i + 1) * BSH] = (
            np.asarray(res.results[i]["outT"]).T.astype(np.float32)
        )
    return out


# revision 16
# speedup vs baseline: 1.1631x; 1.1631x over previous
"""Benes butterfly network (12 layers, N=4096) on 8 Trainium2 NeuronCores.

Self-contained: takes full inputs, shards batch across 8 cores, runs a
Bass/Tile kernel per core, gathers the full output.

Math: reference layer k is a butterfly with span 2^k:
    h[:, j] <- A_k[j] * h[:, j] + B_k[j] * h[:, j ^ 2^k]
(A_k/B_k extracted from the sparse COO (values, idx_in, idx_out)).

V4 design. The device works column-major end to end (the input is
transposed on the host, and the output transpose joins the host gather,
killing the prior version's 28us PE transpose phase):
  - Layers 0..8 composed on the host into dense 128x128 block matrices
    (mst, bf16) with diag(A9*A10*A11) folded in:
      p1[t] = sum_{ji<4} M[t, t^ji] @ H0[t^ji]    (PE, N=512, fp32 psum)
  - Layer 9 ON THE PE: after ACT evacuates E=bf16(p1) (fused [128,1024]
    pair copies), one diagonal matmul per tile accumulates the partner
    term onto the still-resident psum (start=False):
      psum[t] += diag(s9[t]) @ E[t^4]     ( -> Y[t] = A10*A11*H9[t] )
    and a second fused ACT evacuation captures Y in bf16. L9 costs the
    DVE nothing; the measured op menu made DVE the binding engine.
  - Layers 10/11 as fused scalar_tensor_tensor on DVE (664ns measured;
    decomposed mul+add chains inflate ~30% from pipe-drain stalls, and
    ANY concurrent GPSIMD tensor work slows every DVE op ~17% via the
    shared SBUF port -- so: fused stt, GPSIMD kept off tensor work):
      Z[t]   = Y[t] + s10[t] * Y[t^8]     ( = A11*H10[t] )
      H11[t] = Z[t] + r11[t] * Z[t^16]    ( exact )
  - Output written column-major (outT [N, BSH] bf16, contiguous 1KB
    lines); the host gather transposes + upcasts.
  - Jobs (residue, m-pair) flat-ordered to interleave residues 0/1
    during the xT load ramp; paced loads issued from the otherwise-idle
    GPSIMD queue, gated on chunk arrivals by tiny trigger copies (the
    ~630ns/issue DIRECT2D dispatch would otherwise eat ACT sequencer
    time); warmup matmuls hold the PE busy through the ramp so HAM
    reaches K=8/8 before the real matmuls.
"""
import os
import numpy as np
import ml_dtypes

N = 4096
BATCH = 4096
NLAYERS = 12
NCORES = 8
BSH = BATCH // NCORES      # 512 batch rows per core
T = N // 128               # 32 column tiles

_PROGRAM_CACHE = {}
LAST_EXEC_NS = None


def _extract_ab(values, idx_in, idx_out):
    """Per-layer butterfly coefficients A[k], B[k] (float64 [L, N])."""
    v = np.asarray(values, np.float64)
    ii = np.asarray(idx_in, np.int64)
    io = np.asarray(idx_out, np.int64)
    L, nnz = v.shape
    n = nnz // 2
    A = np.zeros((L, n))
    B = np.zeros((L, n))
    for k in range(L):
        s = 1 << k
        self_m = ii[k] == io[k]
        part_m = ii[k] == (io[k] ^ s)
        if not np.all(self_m | part_m):
            raise ValueError(f"layer {k}: unexpected sparse index structure")
        np.add.at(A[k], io[k][self_m], v[k][self_m])
        np.add.at(B[k], io[k][part_m], v[k][part_m])
    return A, B


def _clamp(a):
    return np.where(np.abs(a) < 1e-12, 1e-12, a)


def _host_precompute(values, idx_in, idx_out):
    A, B = _extract_ab(values, idx_in, idx_out)
    Ab = A.reshape(NLAYERS, T, 128)
    Bb = B.reshape(NLAYERS, T, 128)
    j = np.arange(128)

    # Dense composition of within-block layers 0..6, one 128x128 per tile.
    S = [np.eye(128) for _ in range(T)]
    for k in range(7):
        s = 1 << k
        for t in range(T):
            W = np.zeros((128, 128))
            W[j, j] = Ab[k, t]
            W[j, j ^ s] = Bb[k, t]
            S[t] = W @ S[t]
    # Cross-block layers 7, 8 (tile distances 1, 2): dict src_tile -> 128x128
    Sd = [{t: S[t]} for t in range(T)]
    for k in (7, 8):
        d = 1 << (k - 7)
        newS = []
        for t in range(T):
            out = {}
            for src, M in Sd[t].items():
                out[src] = Ab[k, t][:, None] * M
            for src, M in Sd[t ^ d].items():
                out[src] = out.get(src, 0) + Bb[k, t][:, None] * M
            newS.append(out)
        Sd = newS

    D = Ab[9] * Ab[10] * Ab[11]          # folded diagonal, [T, 128]

    # mst (bf16): column block for (r, m) at (r*8 + m)*512, 4 ji-blocks of
    # (diag(D[t]) @ Sd[t][t^ji]).T so matmul computes M @ H0.
    mst = np.zeros((128, T * 512), np.float32)
    for r in range(4):
        for m in range(8):
            t = 4 * m + r
            assert set(Sd[t].keys()) == {t, t ^ 1, t ^ 2, t ^ 3}
            base = (r * 8 + m) * 512
            for ji in range(4):
                M = D[t][:, None] * Sd[t][t ^ ji]
                mst[:, base + ji * 128: base + (ji + 1) * 128] = (
                    M.T.astype(np.float32)
                )

    # combine scalar ratios
    s9 = np.zeros((T, 128))
    stabf = np.zeros((128, 96), np.float64)
    for t in range(T):
        s9[t] = Ab[10, t] * Ab[11, t] * Bb[9, t] / _clamp(D[t ^ 4])
        stabf[:, 32 + t] = (
            Ab[11, t] * Bb[10, t] / _clamp(Ab[10, t ^ 8] * Ab[11, t ^ 8])
        )
        stabf[:, 64 + t] = Bb[11, t] / _clamp(Ab[11, t ^ 16])

    # L9 partner diagonals for the PE accumulate matmul, residue-major:
    # block (r*8 + m) holds diag(s9[4m+r])
    mstd = np.zeros((128, T * 128), np.float32)
    for r in range(4):
        for m in range(8):
            t = 4 * m + r
            mstd[j, (r * 8 + m) * 128 + j] = s9[t]

    return (mst.astype(ml_dtypes.bfloat16), mstd.astype(ml_dtypes.bfloat16),
            stabf.astype(np.float32))


# Flat job order: (residue, m-pair); residues 0/1 interleaved so the PE
# has runnable work while the tail of xT is still loading.
_JOBS = [
    (0, 0), (0, 1), (1, 0), (0, 2), (1, 1), (0, 3), (1, 2), (1, 3),
    (2, 0), (2, 1), (2, 2), (2, 3), (3, 0), (3, 1), (3, 2), (3, 3),
]


def _build_program():
    import concourse.mybir as mybir
    import concourse.tile as tile
    from concourse import bacc

    f32 = mybir.dt.float32
    bf16 = mybir.dt.bfloat16
    mult = mybir.AluOpType.mult
    add = mybir.AluOpType.add

    nc = bacc.Bacc("TRN2", target_bir_lowering=False, debug=False)
    # xT pre-shuffled on host to each H0 tile's exact SBUF layout: row
    # m*128+p holds [lt, b] contiguously -> 4KB descriptors.
    xT_ap = nc.dram_tensor("xT", [8 * 128, 2048], bf16, kind="ExternalInput").ap()
    mst_ap = nc.dram_tensor("mst", [128, T * 512], bf16, kind="ExternalInput").ap()
    mstd_ap = nc.dram_tensor("mstd", [128, T * 128], bf16, kind="ExternalInput").ap()
    stabf_ap = nc.dram_tensor("stabf", [128, 96], f32, kind="ExternalInput").ap()
    out_ap = nc.dram_tensor("outT", [N, BSH], bf16, kind="ExternalOutput").ap()

    with tile.TileContext(nc) as tc:
        with (
            tc.tile_pool(name="const", bufs=1) as constp,
            tc.tile_pool(name="h0", bufs=8) as h0p,
            tc.tile_pool(name="mstp", bufs=8) as mstp,
            tc.tile_pool(name="mstdp", bufs=4) as mstdp,
            tc.tile_pool(name="e2", bufs=5) as e2p,
            tc.tile_pool(name="y2", bufs=10) as y2p,
            tc.tile_pool(name="z", bufs=24) as zp,
            tc.tile_pool(name="h11", bufs=14) as h11p,
            tc.tile_pool(name="trg", bufs=2) as trgp,
            tc.tile_pool(name="ps", bufs=3, space="PSUM") as psp,
            tc.tile_pool(name="pw", bufs=1, space="PSUM") as pwp,
        ):
            stabf = constp.tile([128, 96], f32, name="stabf")

            msth = {}

            def issue_mst(r, half, eng):
                tl = mstp.tile(
                    [128, 2048], bf16, tag="mst", name=f"mst_{r}{half}"
                )
                eng.dma_start(
                    tl[:],
                    mst_ap[:, r * 4096 + half * 2048:
                           r * 4096 + (half + 1) * 2048],
                )
                msth[(r, half)] = tl

            def mst_slice(r, m, ji):
                tl = msth[(r, m // 4)]
                mm = m % 4
                return tl[:, mm * 512 + ji * 128: mm * 512 + (ji + 1) * 128]

            mstdh = {}

            def issue_mstd(r, eng):
                tl = mstdp.tile([128, 1024], bf16, tag="mstd", name=f"mstd_{r}")
                eng.dma_start(tl[:], mstd_ap[:, r * 1024:(r + 1) * 1024])
                mstdh[r] = tl

            def mstd_slice(r, m):
                return mstdh[r][:, m * 128:(m + 1) * 128]

            H0 = {}

            def issue_h0(m, eng):
                tl = h0p.tile([128, 2048], bf16, tag="h0", name=f"h0_{m}")
                eng.dma_start(tl[:], xT_ap[m * 128:(m + 1) * 128, :])
                H0[m] = tl

            # upfront, in need-order on one HWDGE ring (FIFO per ring)
            nc.sync.dma_start(stabf[:], stabf_ap[:])
            issue_mst(0, 0, nc.sync)
            issue_h0(0, nc.sync)
            issue_h0(1, nc.sync)
            issue_mstd(0, nc.sync)

            # warmup matmuls (need only stabf): hold the PE busy through
            # the load ramp so HAM reaches K=8/8 before real matmuls
            warm = pwp.tile([64, 96], f32, tag="warm", name="warm")
            for _ in range(28):
                nc.tensor.matmul(
                    warm[:], stabf[:, 0:64], stabf[:, 0:96],
                    start=True, stop=True,
                )

            # paced loads: issued from the idle GPSIMD queue, gated on an
            # earlier chunk's ARRIVAL by a tiny trigger copy (keeps the
            # ~630ns DIRECT2D dispatches off the busy ACT sequencer)
            def trig_then(gate_tile, thunks):
                trg = trgp.tile([128, 8], bf16, tag="trg", name="trg")
                nc.gpsimd.tensor_copy(trg[:], gate_tile[:, 0:8])
                for th in thunks:
                    th()

            trig_then(H0[0], [
                lambda: issue_h0(2, nc.gpsimd),
                lambda: issue_h0(3, nc.gpsimd),
                lambda: issue_mst(1, 0, nc.gpsimd),
            ])
            trig_then(H0[2], [
                lambda: issue_h0(4, nc.gpsimd),
                lambda: issue_h0(5, nc.gpsimd),
                lambda: issue_mst(0, 1, nc.gpsimd),
                lambda: issue_mstd(1, nc.gpsimd),
            ])
            trig_then(H0[4], [
                lambda: issue_h0(6, nc.gpsimd),
                lambda: issue_h0(7, nc.gpsimd),
                lambda: issue_mst(1, 1, nc.gpsimd),
            ])
            trig_then(H0[6], [
                lambda: issue_mst(2, 0, nc.gpsimd),
                lambda: issue_mst(2, 1, nc.gpsimd),
                lambda: issue_mstd(2, nc.gpsimd),
                lambda: issue_mst(3, 0, nc.gpsimd),
                lambda: issue_mst(3, 1, nc.gpsimd),
                lambda: issue_mstd(3, nc.gpsimd),
            ])

            E2 = {}      # (r, mp) -> [128,1024] bf16: p1 of tiles (t, t^4)
            Y2 = {}      # (r, mp) -> [128,1024] bf16: Y of tiles (t, t^4)
            Z = {}
            H11 = {}
            P2 = {}      # live psum pair tiles
            pend11 = []  # deferred L11 work items (r, m)

            def Ecol(r, m):
                return E2[(r, m >> 1)][:, (m & 1) * 512:((m & 1) + 1) * 512]

            def Ycol(r, m):
                return Y2[(r, m >> 1)][:, (m & 1) * 512:((m & 1) + 1) * 512]

            def emit_l11(r, m):
                t = 4 * m + r
                H11[t] = h11p.tile([128, BSH], bf16, tag="h11", name=f"H11_{t}")
                nc.vector.scalar_tensor_tensor(
                    H11[t][:], Z[4 * (m ^ 4) + r][:],
                    stabf[:, 64 + t:64 + t + 1], Z[t][:],
                    op0=mult, op1=add,
                )
                nc.sync.dma_start(out_ap[t * 128:(t + 1) * 128, :], H11[t][:])

            def emit_l9_and_y(r, mp):
                """PE accumulates diag(s9)@E[partner] onto the resident
                psum pair, then ACT evacuates Y (fused)."""
                for h, m in enumerate((2 * mp, 2 * mp + 1)):
                    nc.tensor.matmul(
                        P2[(r, mp)][:, h * 512:(h + 1) * 512],
                        mstd_slice(r, m),
                        Ecol(r, m ^ 1),
                        start=False, stop=True,
                    )
                y2 = y2p.tile([128, 1024], bf16, tag="y2", name=f"y2_{r}_{mp}")
                nc.scalar.copy(y2[:], P2[(r, mp)][:])
                Y2[(r, mp)] = y2

            def emit_l10(r, ms):
                for m in ms:
                    t = 4 * m + r
                    Z[t] = zp.tile([128, BSH], bf16, tag="z", name=f"Z_{t}")
                    nc.vector.scalar_tensor_tensor(
                        Z[t][:], Ycol(r, m ^ 2),
                        stabf[:, 32 + t:32 + t + 1], Ycol(r, m),
                        op0=mult, op1=add,
                    )

            prev_job = None
            for idx, (r, mp) in enumerate(_JOBS):
                # ---- phase 1: 8 dense matmuls into a [128,1024] psum pair
                p2 = psp.tile([128, 1024], f32, tag="ps", name=f"p2_{r}_{mp}")
                P2[(r, mp)] = p2
                for h, m in enumerate((2 * mp, 2 * mp + 1)):
                    for ji in range(4):
                        nc.tensor.matmul(
                            p2[:, h * 512:(h + 1) * 512],
                            mst_slice(r, m, ji),
                            H0[m][:, (r ^ ji) * 512:((r ^ ji) + 1) * 512],
                            start=(ji == 0), stop=False,
                        )
                # ---- fused ACT evacuation E (psum pair stays resident)
                e2 = e2p.tile([128, 1024], bf16, tag="e2", name=f"e2_{r}_{mp}")
                nc.scalar.copy(e2[:], p2[:])
                E2[(r, mp)] = e2

                # ---- previous job's L9 diag-matmuls + Y evacuation
                # (deferred one job so the PE never waits on the evac)
                if prev_job is not None:
                    pr, pmp = prev_job
                    emit_l9_and_y(pr, pmp)
                    if pmp == 1:
                        emit_l10(pr, range(0, 4))
                    if pmp == 3:
                        emit_l10(pr, range(4, 8))
                        pend11.extend((pr, m) for m in range(8))
                prev_job = (r, mp)

                # ---- drain deferred L11 steps (DVE) + their stores
                for _ in range(3 if idx >= 10 else 2):
                    if pend11:
                        p_r, p_m = pend11.pop(0)
                        emit_l11(p_r, p_m)

            # tail: last job's L9/Y, last L10 half, remaining L11
            pr, pmp = prev_job
            emit_l9_and_y(pr, pmp)
            emit_l10(pr, range(4, 8))
            pend11.extend((pr, m) for m in range(8))
            for p_r, p_m in pend11:
                emit_l11(p_r, p_m)

    nc.compile()
    return nc


def kernel(x, values, idx_in, idx_out):
    global LAST_EXEC_NS
    from concourse.bass_utils import run_bass_kernel_spmd

    x = np.asarray(x, np.float32)
    assert x.shape == (BATCH, N), x.shape
    mst, mstd, stabf = _host_precompute(values, idx_in, idx_out)
    xT = np.ascontiguousarray(x.T.astype(ml_dtypes.bfloat16))

    if "prog" not in _PROGRAM_CACHE:
        _PROGRAM_CACHE["prog"] = _build_program()
    nc = _PROGRAM_CACHE["prog"]

    in_maps = []
    for i in range(NCORES):
        xc = xT[:, i * BSH:(i + 1) * BSH]                  # [N, 512]
        # row m*128+p holds (lt, b) contiguously = H0 tile SBUF layout
        xs = np.ascontiguousarray(
            xc.reshape(8, 4, 128, BSH).transpose(0, 2, 1, 3).reshape(1024, 2048)
        )
        in_maps.append({"xT": xs, "mst": mst, "mstd": mstd, "stabf": stabf})
    res = run_bass_kernel_spmd(nc, in_maps, core_ids=list(range(NCORES)))
    if os.environ.get("BENES_TRACE"):
        tres = run_bass_kernel_spmd(
            nc, in_maps, core_ids=list(range(NCORES)), trace=True
        )
        LAST_EXEC_NS = tres.exec_time_ns
        _PROGRAM_CACHE["profile_json"] = tres.profile_json
    out = np.empty((BATCH, N), np.float32)
    for i in range(NCORES):
        out[i * BSH:(i + 1) * BSH] = (
            np.asarray(res.results[i]["outT"]).T.astype(np.float32)
        )
    return out


# revision 18
# speedup vs baseline: 1.1805x; 1.0150x over previous
"""Benes butterfly network (12 layers, N=4096) on 8 Trainium2 NeuronCores.

Self-contained: takes full inputs, shards batch across 8 cores, runs a
Bass/Tile kernel per core, gathers the full output.

Math: reference layer k is a butterfly with span 2^k:
    h[:, j] <- A_k[j] * h[:, j] + B_k[j] * h[:, j ^ 2^k]
(A_k/B_k extracted from the sparse COO (values, idx_in, idx_out)).

V4 design. The device works column-major end to end (the input is
transposed on the host, and the output transpose joins the host gather,
killing the prior version's 28us PE transpose phase):
  - Layers 0..8 composed on the host into dense 128x128 block matrices
    (mst, bf16) with diag(A9*A10*A11) folded in:
      p1[t] = sum_{ji<4} M[t, t^ji] @ H0[t^ji]    (PE, N=512, fp32 psum)
  - Layer 9 ON THE PE: after ACT evacuates E=bf16(p1) (fused [128,1024]
    pair copies), one diagonal matmul per tile accumulates the partner
    term onto the still-resident psum (start=False):
      psum[t] += diag(s9[t]) @ E[t^4]     ( -> Y[t] = A10*A11*H9[t] )
    and a second fused ACT evacuation captures Y in bf16. L9 costs the
    DVE nothing; the measured op menu made DVE the binding engine.
  - Layers 10/11 as fused scalar_tensor_tensor on DVE (664ns measured;
    decomposed mul+add chains inflate ~30% from pipe-drain stalls, and
    ANY concurrent GPSIMD tensor work slows every DVE op ~17% via the
    shared SBUF port -- so: fused stt, GPSIMD kept off tensor work):
      Z[t]   = Y[t] + s10[t] * Y[t^8]     ( = A11*H10[t] )
      H11[t] = Z[t] + r11[t] * Z[t^16]    ( exact )
  - Output written column-major (outT [N, BSH] bf16, contiguous 1KB
    lines); the host gather transposes + upcasts.
  - Jobs (residue, m-pair) flat-ordered to interleave residues 0/1
    during the xT load ramp; paced loads issued from the otherwise-idle
    GPSIMD queue, gated on chunk arrivals by tiny trigger copies (the
    ~630ns/issue DIRECT2D dispatch would otherwise eat ACT sequencer
    time); warmup matmuls hold the PE busy through the ramp so HAM
    reaches K=8/8 before the real matmuls.
"""
import os
import numpy as np
import ml_dtypes

N = 4096
BATCH = 4096
NLAYERS = 12
NCORES = 8
BSH = BATCH // NCORES      # 512 batch rows per core
T = N // 128               # 32 column tiles

_PROGRAM_CACHE = {}
LAST_EXEC_NS = None


def _extract_ab(values, idx_in, idx_out):
    """Per-layer butterfly coefficients A[k], B[k] (float64 [L, N])."""
    v = np.asarray(values, np.float64)
    ii = np.asarray(idx_in, np.int64)
    io = np.asarray(idx_out, np.int64)
    L, nnz = v.shape
    n = nnz // 2
    A = np.zeros((L, n))
    B = np.zeros((L, n))
    for k in range(L):
        s = 1 << k
        self_m = ii[k] == io[k]
        part_m = ii[k] == (io[k] ^ s)
        if not np.all(self_m | part_m):
            raise ValueError(f"layer {k}: unexpected sparse index structure")
        np.add.at(A[k], io[k][self_m], v[k][self_m])
        np.add.at(B[k], io[k][part_m], v[k][part_m])
    return A, B


def _clamp(a):
    return np.where(np.abs(a) < 1e-12, 1e-12, a)


def _host_precompute(values, idx_in, idx_out):
    A, B = _extract_ab(values, idx_in, idx_out)
    Ab = A.reshape(NLAYERS, T, 128)
    Bb = B.reshape(NLAYERS, T, 128)
    j = np.arange(128)

    # Dense composition of within-block layers 0..6, one 128x128 per tile.
    S = [np.eye(128) for _ in range(T)]
    for k in range(7):
        s = 1 << k
        for t in range(T):
            W = np.zeros((128, 128))
            W[j, j] = Ab[k, t]
            W[j, j ^ s] = Bb[k, t]
            S[t] = W @ S[t]
    # Cross-block layers 7, 8 (tile distances 1, 2): dict src_tile -> 128x128
    Sd = [{t: S[t]} for t in range(T)]
    for k in (7, 8):
        d = 1 << (k - 7)
        newS = []
        for t in range(T):
            out = {}
            for src, M in Sd[t].items():
                out[src] = Ab[k, t][:, None] * M
            for src, M in Sd[t ^ d].items():
                out[src] = out.get(src, 0) + Bb[k, t][:, None] * M
            newS.append(out)
        Sd = newS

    D = Ab[9] * Ab[10] * Ab[11]          # folded diagonal, [T, 128]

    # mst (bf16): column block for (r, m) at (r*8 + m)*512, 4 ji-blocks of
    # (diag(D[t]) @ Sd[t][t^ji]).T so matmul computes M @ H0.
    mst = np.zeros((128, T * 512), np.float32)
    for r in range(4):
        for m in range(8):
            t = 4 * m + r
            assert set(Sd[t].keys()) == {t, t ^ 1, t ^ 2, t ^ 3}
            base = (r * 8 + m) * 512
            for ji in range(4):
                M = D[t][:, None] * Sd[t][t ^ ji]
                mst[:, base + ji * 128: base + (ji + 1) * 128] = (
                    M.T.astype(np.float32)
                )

    # combine scalar ratios
    s9 = np.zeros((T, 128))
    stabf = np.zeros((128, 96), np.float64)
    for t in range(T):
        s9[t] = Ab[10, t] * Ab[11, t] * Bb[9, t] / _clamp(D[t ^ 4])
        stabf[:, 32 + t] = (
            Ab[11, t] * Bb[10, t] / _clamp(Ab[10, t ^ 8] * Ab[11, t ^ 8])
        )
        stabf[:, 64 + t] = Bb[11, t] / _clamp(Ab[11, t ^ 16])

    # L9 partner diagonals for the PE accumulate matmul, residue-major:
    # block (r*8 + m) holds diag(s9[4m+r])
    mstd = np.zeros((128, T * 128), np.float32)
    for r in range(4):
        for m in range(8):
            t = 4 * m + r
            mstd[j, (r * 8 + m) * 128 + j] = s9[t]

    return (mst.astype(ml_dtypes.bfloat16), mstd.astype(ml_dtypes.bfloat16),
            stabf.astype(np.float32))


# Flat job order: (residue, m-pair); residues 0/1 interleaved so the PE
# has runnable work while the tail of xT is still loading.
_JOBS = [
    (0, 0), (0, 1), (1, 0), (0, 2), (1, 1), (0, 3), (1, 2), (1, 3),
    (2, 0), (2, 1), (2, 2), (2, 3), (3, 0), (3, 1), (3, 2), (3, 3),
]


def _build_program():
    import concourse.mybir as mybir
    import concourse.tile as tile
    from concourse import bacc

    f32 = mybir.dt.float32
    bf16 = mybir.dt.bfloat16
    mult = mybir.AluOpType.mult
    add = mybir.AluOpType.add

    nc = bacc.Bacc("TRN2", target_bir_lowering=False, debug=False)
    # xT pre-shuffled on host to each H0 tile's exact SBUF layout: row
    # m*128+p holds [lt, b] contiguously -> 4KB descriptors.
    xT_ap = nc.dram_tensor("xT", [8 * 128, 2048], bf16, kind="ExternalInput").ap()
    mst_ap = nc.dram_tensor("mst", [128, T * 512], bf16, kind="ExternalInput").ap()
    mstd_ap = nc.dram_tensor("mstd", [128, T * 128], bf16, kind="ExternalInput").ap()
    stabf_ap = nc.dram_tensor("stabf", [128, 96], f32, kind="ExternalInput").ap()
    out_ap = nc.dram_tensor("outT", [N, BSH], bf16, kind="ExternalOutput").ap()

    with tile.TileContext(nc) as tc:
        with (
            tc.tile_pool(name="const", bufs=1) as constp,
            tc.tile_pool(name="h0", bufs=8) as h0p,
            tc.tile_pool(name="mstp", bufs=8) as mstp,
            tc.tile_pool(name="mstdp", bufs=4) as mstdp,
            tc.tile_pool(name="e2", bufs=5) as e2p,
            tc.tile_pool(name="y2", bufs=10) as y2p,
            tc.tile_pool(name="z", bufs=24) as zp,
            tc.tile_pool(name="h11", bufs=14) as h11p,
            tc.tile_pool(name="trg", bufs=2) as trgp,
            tc.tile_pool(name="ps", bufs=3, space="PSUM") as psp,
            tc.tile_pool(name="pw", bufs=1, space="PSUM") as pwp,
        ):
            stabf = constp.tile([128, 96], f32, name="stabf")

            msth = {}

            def issue_mst(r, half, eng):
                tl = mstp.tile(
                    [128, 2048], bf16, tag="mst", name=f"mst_{r}{half}"
                )
                eng.dma_start(
                    tl[:],
                    mst_ap[:, r * 4096 + half * 2048:
                           r * 4096 + (half + 1) * 2048],
                )
                msth[(r, half)] = tl

            def mst_slice(r, m, ji):
                tl = msth[(r, m // 4)]
                mm = m % 4
                return tl[:, mm * 512 + ji * 128: mm * 512 + (ji + 1) * 128]

            mstdh = {}

            def issue_mstd(r, eng):
                tl = mstdp.tile([128, 1024], bf16, tag="mstd", name=f"mstd_{r}")
                eng.dma_start(tl[:], mstd_ap[:, r * 1024:(r + 1) * 1024])
                mstdh[r] = tl

            def mstd_slice(r, m):
                return mstdh[r][:, m * 128:(m + 1) * 128]

            H0 = {}

            def issue_h0(m, eng):
                tl = h0p.tile([128, 2048], bf16, tag="h0", name=f"h0_{m}")
                eng.dma_start(tl[:], xT_ap[m * 128:(m + 1) * 128, :])
                H0[m] = tl

            # upfront loads spread across FOUR engine rings so both the
            # transfers and their ~1.5us completion overheads overlap
            # (serialized on one ring, H0[0] measured landing at 16.8us):
            #  - scalar: mst00 as the ACT queue's FIRST instruction (ahead
            #    of the auto-inserted ACT_TABLE_LOAD)
            #  - vector: H0[0], H0[2], H0[3] (DVE idle until ~20us)
            #  - gpsimd: H0[1], then all trig-paced later chunks
            #  - sync:   stabf, mstd0 (stores come later)
            issue_mst(0, 0, nc.scalar)
            issue_h0(0, nc.gpsimd)
            nc.sync.dma_start(stabf[:], stabf_ap[:])
            issue_mstd(0, nc.sync)
            issue_h0(1, nc.sync)
            issue_mst(1, 0, nc.scalar)
            issue_h0(2, nc.gpsimd)
            issue_h0(3, nc.sync)

            # warmup matmuls (need only stabf): bridge the short ramp
            warm = pwp.tile([64, 96], f32, tag="warm", name="warm")
            for _ in range(10):
                nc.tensor.matmul(
                    warm[:], stabf[:, 0:64], stabf[:, 0:96],
                    start=True, stop=True,
                )

            # paced loads from the idle GPSIMD queue, gated on an earlier
            # chunk's ARRIVAL by a tiny trigger copy
            def trig_then(gate_tile, thunks):
                trg = trgp.tile([128, 8], bf16, tag="trg", name="trg")
                nc.gpsimd.tensor_copy(trg[:], gate_tile[:, 0:8])
                for th in thunks:
                    th()

            trig_then(H0[0], [
                lambda: issue_h0(4, nc.gpsimd),
                lambda: issue_h0(5, nc.gpsimd),
            ])
            trig_then(H0[4], [
                lambda: issue_mst(0, 1, nc.gpsimd),
                lambda: issue_mstd(1, nc.gpsimd),
                lambda: issue_h0(6, nc.gpsimd),
                lambda: issue_h0(7, nc.gpsimd),
                lambda: issue_mst(1, 1, nc.gpsimd),
            ])
            trig_then(H0[6], [
                lambda: issue_mst(2, 0, nc.gpsimd),
                lambda: issue_mst(2, 1, nc.gpsimd),
                lambda: issue_mstd(2, nc.gpsimd),
                lambda: issue_mst(3, 0, nc.gpsimd),
                lambda: issue_mst(3, 1, nc.gpsimd),
                lambda: issue_mstd(3, nc.gpsimd),
            ])

            E2 = {}      # (r, mp) -> [128,1024] bf16: p1 of tiles (t, t^4)
            Y2 = {}      # (r, mp) -> [128,1024] bf16: Y of tiles (t, t^4)
            Z = {}
            H11 = {}
            P2 = {}      # live psum pair tiles
            pend11 = []  # deferred L11 work items (r, m)

            def Ecol(r, m):
                return E2[(r, m >> 1)][:, (m & 1) * 512:((m & 1) + 1) * 512]

            def Ycol(r, m):
                return Y2[(r, m >> 1)][:, (m & 1) * 512:((m & 1) + 1) * 512]

            def emit_l11(r, m):
                t = 4 * m + r
                H11[t] = h11p.tile([128, BSH], bf16, tag="h11", name=f"H11_{t}")
                nc.vector.scalar_tensor_tensor(
                    H11[t][:], Z[4 * (m ^ 4) + r][:],
                    stabf[:, 64 + t:64 + t + 1], Z[t][:],
                    op0=mult, op1=add,
                )
                nc.sync.dma_start(out_ap[t * 128:(t + 1) * 128, :], H11[t][:])

            def emit_l9_and_y(r, mp):
                """PE accumulates diag(s9)@E[partner] onto the resident
                psum pair, then ACT evacuates Y (fused)."""
                for h, m in enumerate((2 * mp, 2 * mp + 1)):
                    nc.tensor.matmul(
                        P2[(r, mp)][:, h * 512:(h + 1) * 512],
                        mstd_slice(r, m),
                        Ecol(r, m ^ 1),
                        start=False, stop=True,
                    )
                y2 = y2p.tile([128, 1024], bf16, tag="y2", name=f"y2_{r}_{mp}")
                nc.scalar.copy(y2[:], P2[(r, mp)][:])
                Y2[(r, mp)] = y2

            def emit_l10(r, ms):
                for m in ms:
                    t = 4 * m + r
                    Z[t] = zp.tile([128, BSH], bf16, tag="z", name=f"Z_{t}")
                    nc.vector.scalar_tensor_tensor(
                        Z[t][:], Ycol(r, m ^ 2),
                        stabf[:, 32 + t:32 + t + 1], Ycol(r, m),
                        op0=mult, op1=add,
                    )

            prev_job = None
            for idx, (r, mp) in enumerate(_JOBS):
                # ---- phase 1: 8 dense matmuls into a [128,1024] psum pair
                p2 = psp.tile([128, 1024], f32, tag="ps", name=f"p2_{r}_{mp}")
                P2[(r, mp)] = p2
                for h, m in enumerate((2 * mp, 2 * mp + 1)):
                    for ji in range(4):
                        nc.tensor.matmul(
                            p2[:, h * 512:(h + 1) * 512],
                            mst_slice(r, m, ji),
                            H0[m][:, (r ^ ji) * 512:((r ^ ji) + 1) * 512],
                            start=(ji == 0), stop=False,
                        )
                # ---- fused ACT evacuation E (psum pair stays resident)
                e2 = e2p.tile([128, 1024], bf16, tag="e2", name=f"e2_{r}_{mp}")
                nc.scalar.copy(e2[:], p2[:])
                E2[(r, mp)] = e2

                # ---- previous job's L9 diag-matmuls + Y evacuation
                # (deferred one job so the PE never waits on the evac)
                if prev_job is not None:
                    pr, pmp = prev_job
                    emit_l9_and_y(pr, pmp)
                    if pmp == 1:
                        emit_l10(pr, range(0, 4))
                    if pmp == 3:
                        emit_l10(pr, range(4, 8))
                        pend11.extend((pr, m) for m in range(8))
                prev_job = (r, mp)

                # ---- drain deferred L11 steps (DVE) + their stores
                for _ in range(3 if idx >= 10 else 2):
                    if pend11:
                        p_r, p_m = pend11.pop(0)
                        emit_l11(p_r, p_m)

            # tail: last job's L9/Y, last L10 half, remaining L11
            pr, pmp = prev_job
            emit_l9_and_y(pr, pmp)
            emit_l10(pr, range(4, 8))
            pend11.extend((pr, m) for m in range(8))
            for p_r, p_m in pend11:
                emit_l11(p_r, p_m)

    nc.compile()
    return nc


def kernel(x, values, idx_in, idx_out):
    global LAST_EXEC_NS
    from concourse.bass_utils import run_bass_kernel_spmd

    x = np.asarray(x, np.float32)
    assert x.shape == (BATCH, N), x.shape
    mst, mstd, stabf = _host_precompute(values, idx_in, idx_out)
    xT = np.ascontiguousarray(x.T.astype(ml_dtypes.bfloat16))

    if "prog" not in _PROGRAM_CACHE:
        _PROGRAM_CACHE["prog"] = _build_program()
    nc = _PROGRAM_CACHE["prog"]

    in_maps = []
    for i in range(NCORES):
        xc = xT[:, i * BSH:(i + 1) * BSH]                  # [N, 512]
        # row m*128+p holds (lt, b) contiguously = H0 tile SBUF layout
        xs = np.ascontiguousarray(
            xc.reshape(8, 4, 128, BSH).transpose(0, 2, 1, 3).reshape(1024, 2048)
        )
        in_maps.append({"xT": xs, "mst": mst, "mstd": mstd, "stabf": stabf})
    res = run_bass_kernel_spmd(nc, in_maps, core_ids=list(range(NCORES)))
    if os.environ.get("BENES_TRACE"):
        tres = run_bass_kernel_spmd(
            nc, in_maps, core_ids=list(range(NCORES)), trace=True
        )
        LAST_EXEC_NS = tres.exec_time_ns
        _PROGRAM_CACHE["profile_json"] = tres.profile_json
    out = np.empty((BATCH, N), np.float32)
    for i in range(NCORES):
        out[i * BSH:(i + 1) * BSH] = (
            np.asarray(res.results[i]["outT"]).T.astype(np.float32)
        )
    return out


# revision 19
# speedup vs baseline: 1.2014x; 1.0177x over previous
"""Benes butterfly network (12 layers, N=4096) on 8 Trainium2 NeuronCores.

Self-contained: takes full inputs, shards batch across 8 cores, runs a
Bass/Tile kernel per core, gathers the full output.

Math: reference layer k is a butterfly with span 2^k:
    h[:, j] <- A_k[j] * h[:, j] + B_k[j] * h[:, j ^ 2^k]
(A_k/B_k extracted from the sparse COO (values, idx_in, idx_out)).

V4 design. The device works column-major end to end (the input is
transposed on the host, and the output transpose joins the host gather,
killing the prior version's 28us PE transpose phase):
  - Layers 0..8 composed on the host into dense 128x128 block matrices
    (mst, bf16) with diag(A9*A10*A11) folded in:
      p1[t] = sum_{ji<4} M[t, t^ji] @ H0[t^ji]    (PE, N=512, fp32 psum)
  - Layer 9 ON THE PE: after ACT evacuates E=bf16(p1) (fused [128,1024]
    pair copies), one diagonal matmul per tile accumulates the partner
    term onto the still-resident psum (start=False):
      psum[t] += diag(s9[t]) @ E[t^4]     ( -> Y[t] = A10*A11*H9[t] )
    and a second fused ACT evacuation captures Y in bf16. L9 costs the
    DVE nothing; the measured op menu made DVE the binding engine.
  - Layers 10/11 as fused scalar_tensor_tensor on DVE (664ns measured;
    decomposed mul+add chains inflate ~30% from pipe-drain stalls, and
    ANY concurrent GPSIMD tensor work slows every DVE op ~17% via the
    shared SBUF port -- so: fused stt, GPSIMD kept off tensor work):
      Z[t]   = Y[t] + s10[t] * Y[t^8]     ( = A11*H10[t] )
      H11[t] = Z[t] + r11[t] * Z[t^16]    ( exact )
  - Output written column-major (outT [N, BSH] bf16, contiguous 1KB
    lines); the host gather transposes + upcasts.
  - Jobs (residue, m-pair) flat-ordered to interleave residues 0/1
    during the xT load ramp; paced loads issued from the otherwise-idle
    GPSIMD queue, gated on chunk arrivals by tiny trigger copies (the
    ~630ns/issue DIRECT2D dispatch would otherwise eat ACT sequencer
    time); warmup matmuls hold the PE busy through the ramp so HAM
    reaches K=8/8 before the real matmuls.
"""
import os
import numpy as np
import ml_dtypes

N = 4096
BATCH = 4096
NLAYERS = 12
NCORES = 8
BSH = BATCH // NCORES      # 512 batch rows per core
T = N // 128               # 32 column tiles

_PROGRAM_CACHE = {}
LAST_EXEC_NS = None


def _extract_ab(values, idx_in, idx_out):
    """Per-layer butterfly coefficients A[k], B[k] (float64 [L, N])."""
    v = np.asarray(values, np.float64)
    ii = np.asarray(idx_in, np.int64)
    io = np.asarray(idx_out, np.int64)
    L, nnz = v.shape
    n = nnz // 2
    A = np.zeros((L, n))
    B = np.zeros((L, n))
    for k in range(L):
        s = 1 << k
        self_m = ii[k] == io[k]
        part_m = ii[k] == (io[k] ^ s)
        if not np.all(self_m | part_m):
            raise ValueError(f"layer {k}: unexpected sparse index structure")
        np.add.at(A[k], io[k][self_m], v[k][self_m])
        np.add.at(B[k], io[k][part_m], v[k][part_m])
    return A, B


def _clamp(a):
    return np.where(np.abs(a) < 1e-12, 1e-12, a)


def _host_precompute(values, idx_in, idx_out):
    A, B = _extract_ab(values, idx_in, idx_out)
    Ab = A.reshape(NLAYERS, T, 128)
    Bb = B.reshape(NLAYERS, T, 128)
    j = np.arange(128)

    # Dense composition of within-block layers 0..6, one 128x128 per tile.
    S = [np.eye(128) for _ in range(T)]
    for k in range(7):
        s = 1 << k
        for t in range(T):
            W = np.zeros((128, 128))
            W[j, j] = Ab[k, t]
            W[j, j ^ s] = Bb[k, t]
            S[t] = W @ S[t]
    # Cross-block layers 7, 8 (tile distances 1, 2): dict src_tile -> 128x128
    Sd = [{t: S[t]} for t in range(T)]
    for k in (7, 8):
        d = 1 << (k - 7)
        newS = []
        for t in range(T):
            out = {}
            for src, M in Sd[t].items():
                out[src] = Ab[k, t][:, None] * M
            for src, M in Sd[t ^ d].items():
                out[src] = out.get(src, 0) + Bb[k, t][:, None] * M
            newS.append(out)
        Sd = newS

    D = Ab[9] * Ab[10] * Ab[11]          # folded diagonal, [T, 128]

    # mst (bf16): column block for (r, m) at (r*8 + m)*512, 4 ji-blocks of
    # (diag(D[t]) @ Sd[t][t^ji]).T so matmul computes M @ H0.
    mst = np.zeros((128, T * 512), np.float32)
    for r in range(4):
        for m in range(8):
            t = 4 * m + r
            assert set(Sd[t].keys()) == {t, t ^ 1, t ^ 2, t ^ 3}
            base = (r * 8 + m) * 512
            for ji in range(4):
                M = D[t][:, None] * Sd[t][t ^ ji]
                mst[:, base + ji * 128: base + (ji + 1) * 128] = (
                    M.T.astype(np.float32)
                )

    # combine scalar ratios
    s9 = np.zeros((T, 128))
    stabf = np.zeros((128, 96), np.float64)
    for t in range(T):
        s9[t] = Ab[10, t] * Ab[11, t] * Bb[9, t] / _clamp(D[t ^ 4])
        stabf[:, 32 + t] = (
            Ab[11, t] * Bb[10, t] / _clamp(Ab[10, t ^ 8] * Ab[11, t ^ 8])
        )
        stabf[:, 64 + t] = Bb[11, t] / _clamp(Ab[11, t ^ 16])

    # L9 partner diagonals for the PE accumulate matmul, residue-major:
    # block (r*8 + m) holds diag(s9[4m+r])
    mstd = np.zeros((128, T * 128), np.float32)
    for r in range(4):
        for m in range(8):
            t = 4 * m + r
            mstd[j, (r * 8 + m) * 128 + j] = s9[t]

    return (mst.astype(ml_dtypes.bfloat16), mstd.astype(ml_dtypes.bfloat16),
            stabf.astype(np.float32))


# Flat job order: (residue, m-pair); residues 0/1 interleaved so the PE
# has runnable work while the tail of xT is still loading.
_JOBS = [
    (0, 0), (0, 1), (1, 0), (0, 2), (1, 1), (0, 3), (1, 2), (1, 3),
    (2, 0), (2, 1), (2, 2), (2, 3), (3, 0), (3, 1), (3, 2), (3, 3),
]


def _build_program():
    import concourse.mybir as mybir
    import concourse.tile as tile
    from concourse import bacc

    f32 = mybir.dt.float32
    bf16 = mybir.dt.bfloat16
    mult = mybir.AluOpType.mult
    add = mybir.AluOpType.add

    nc = bacc.Bacc("TRN2", target_bir_lowering=False, debug=False)
    # xT pre-shuffled on host to each H0 tile's exact SBUF layout: row
    # m*128+p holds [lt, b] contiguously -> 4KB descriptors.
    xT_ap = nc.dram_tensor("xT", [8 * 128, 2048], bf16, kind="ExternalInput").ap()
    mst_ap = nc.dram_tensor("mst", [128, T * 512], bf16, kind="ExternalInput").ap()
    mstd_ap = nc.dram_tensor("mstd", [128, T * 128], bf16, kind="ExternalInput").ap()
    stabf_ap = nc.dram_tensor("stabf", [128, 96], f32, kind="ExternalInput").ap()
    out_ap = nc.dram_tensor("outT", [N, BSH], bf16, kind="ExternalOutput").ap()

    with tile.TileContext(nc) as tc:
        with (
            tc.tile_pool(name="const", bufs=1) as constp,
            tc.tile_pool(name="h0", bufs=8) as h0p,
            tc.tile_pool(name="mstp", bufs=8) as mstp,
            tc.tile_pool(name="mstdp", bufs=4) as mstdp,
            tc.tile_pool(name="e2", bufs=5) as e2p,
            tc.tile_pool(name="y2", bufs=10) as y2p,
            tc.tile_pool(name="z", bufs=24) as zp,
            tc.tile_pool(name="h11", bufs=14) as h11p,
            tc.tile_pool(name="trg", bufs=2) as trgp,
            tc.tile_pool(name="ps", bufs=3, space="PSUM") as psp,
            tc.tile_pool(name="pw", bufs=1, space="PSUM") as pwp,
        ):
            stabf = constp.tile([128, 96], f32, name="stabf")

            msth = {}

            def issue_mst(r, half, eng):
                tl = mstp.tile(
                    [128, 2048], bf16, tag="mst", name=f"mst_{r}{half}"
                )
                eng.dma_start(
                    tl[:],
                    mst_ap[:, r * 4096 + half * 2048:
                           r * 4096 + (half + 1) * 2048],
                )
                msth[(r, half)] = tl

            def mst_slice(r, m, ji):
                tl = msth[(r, m // 4)]
                mm = m % 4
                return tl[:, mm * 512 + ji * 128: mm * 512 + (ji + 1) * 128]

            mstdh = {}

            def issue_mstd(r, eng):
                tl = mstdp.tile([128, 1024], bf16, tag="mstd", name=f"mstd_{r}")
                eng.dma_start(tl[:], mstd_ap[:, r * 1024:(r + 1) * 1024])
                mstdh[r] = tl

            def mstd_slice(r, m):
                return mstdh[r][:, m * 128:(m + 1) * 128]

            H0 = {}

            def issue_h0(m, eng):
                tl = h0p.tile([128, 2048], bf16, tag="h0", name=f"h0_{m}")
                eng.dma_start(tl[:], xT_ap[m * 128:(m + 1) * 128, :])
                H0[m] = tl

            # upfront loads spread across FOUR engine rings so both the
            # transfers and their ~1.5us completion overheads overlap
            # (serialized on one ring, H0[0] measured landing at 16.8us):
            #  - scalar: mst00 as the ACT queue's FIRST instruction (ahead
            #    of the auto-inserted ACT_TABLE_LOAD)
            #  - vector: H0[0], H0[2], H0[3] (DVE idle until ~20us)
            #  - gpsimd: H0[1], then all trig-paced later chunks
            #  - sync:   stabf, mstd0 (stores come later)
            nc.scalar.dma_start(stabf[:], stabf_ap[:])
            issue_mst(0, 0, nc.scalar)
            issue_h0(1, nc.sync)
            issue_h0(0, nc.gpsimd)
            issue_mst(1, 0, nc.scalar)
            issue_h0(3, nc.sync)
            issue_mstd(0, nc.sync)
            issue_h0(2, nc.gpsimd)

            # warmup matmuls (need only stabf): bridge the short ramp
            warm = pwp.tile([64, 96], f32, tag="warm", name="warm")
            for _ in range(10):
                nc.tensor.matmul(
                    warm[:], stabf[:, 0:64], stabf[:, 0:96],
                    start=True, stop=True,
                )

            # paced loads from the idle GPSIMD queue, gated on an earlier
            # chunk's ARRIVAL by a tiny trigger copy
            def trig_then(gate_tile, thunks):
                trg = trgp.tile([128, 8], bf16, tag="trg", name="trg")
                nc.gpsimd.tensor_copy(trg[:], gate_tile[:, 0:8])
                for th in thunks:
                    th()

            trig_then(H0[0], [
                lambda: issue_h0(4, nc.gpsimd),
                lambda: issue_h0(5, nc.gpsimd),
            ])
            trig_then(H0[4], [
                lambda: issue_mst(0, 1, nc.gpsimd),
                lambda: issue_mstd(1, nc.gpsimd),
                lambda: issue_h0(6, nc.gpsimd),
                lambda: issue_h0(7, nc.gpsimd),
                lambda: issue_mst(1, 1, nc.gpsimd),
            ])
            trig_then(H0[6], [
                lambda: issue_mst(2, 0, nc.gpsimd),
                lambda: issue_mst(2, 1, nc.gpsimd),
                lambda: issue_mstd(2, nc.gpsimd),
                lambda: issue_mst(3, 0, nc.gpsimd),
                lambda: issue_mst(3, 1, nc.gpsimd),
                lambda: issue_mstd(3, nc.gpsimd),
            ])

            E2 = {}      # (r, mp) -> [128,1024] bf16: p1 of tiles (t, t^4)
            Y2 = {}      # (r, mp) -> [128,1024] bf16: Y of tiles (t, t^4)
            Z = {}
            H11 = {}
            P2 = {}      # live psum pair tiles
            pend11 = []  # deferred L11 work items (r, m)

            def Ecol(r, m):
                return E2[(r, m >> 1)][:, (m & 1) * 512:((m & 1) + 1) * 512]

            def Ycol(r, m):
                return Y2[(r, m >> 1)][:, (m & 1) * 512:((m & 1) + 1) * 512]

            def emit_l11(r, m):
                t = 4 * m + r
                H11[t] = h11p.tile([128, BSH], bf16, tag="h11", name=f"H11_{t}")
                nc.vector.scalar_tensor_tensor(
                    H11[t][:], Z[4 * (m ^ 4) + r][:],
                    stabf[:, 64 + t:64 + t + 1], Z[t][:],
                    op0=mult, op1=add,
                )
                nc.sync.dma_start(out_ap[t * 128:(t + 1) * 128, :], H11[t][:])

            def emit_l9_and_y(r, mp):
                """PE accumulates diag(s9)@E[partner] onto the resident
                psum pair, then ACT evacuates Y (fused)."""
                for h, m in enumerate((2 * mp, 2 * mp + 1)):
                    nc.tensor.matmul(
                        P2[(r, mp)][:, h * 512:(h + 1) * 512],
                        mstd_slice(r, m),
                        Ecol(r, m ^ 1),
                        start=False, stop=True,
                    )
                y2 = y2p.tile([128, 1024], bf16, tag="y2", name=f"y2_{r}_{mp}")
                nc.scalar.copy(y2[:], P2[(r, mp)][:])
                Y2[(r, mp)] = y2

            def emit_l10(r, ms):
                for m in ms:
                    t = 4 * m + r
                    Z[t] = zp.tile([128, BSH], bf16, tag="z", name=f"Z_{t}")
                    nc.vector.scalar_tensor_tensor(
                        Z[t][:], Ycol(r, m ^ 2),
                        stabf[:, 32 + t:32 + t + 1], Ycol(r, m),
                        op0=mult, op1=add,
                    )

            prev_job = None
            for idx, (r, mp) in enumerate(_JOBS):
                # ---- phase 1: 8 dense matmuls into a [128,1024] psum pair
                p2 = psp.tile([128, 1024], f32, tag="ps", name=f"p2_{r}_{mp}")
                P2[(r, mp)] = p2
                for h, m in enumerate((2 * mp, 2 * mp + 1)):
                    for ji in range(4):
                        nc.tensor.matmul(
                            p2[:, h * 512:(h + 1) * 512],
                            mst_slice(r, m, ji),
                            H0[m][:, (r ^ ji) * 512:((r ^ ji) + 1) * 512],
                            start=(ji == 0), stop=False,
                        )
                # ---- fused ACT evacuation E (psum pair stays resident)
                e2 = e2p.tile([128, 1024], bf16, tag="e2", name=f"e2_{r}_{mp}")
                nc.scalar.copy(e2[:], p2[:])
                E2[(r, mp)] = e2

                # ---- L9 diag-matmuls + Y evacuation: deferred one job so
                # the PE never waits on the evac -- except the first two
                # jobs, emitted inline (the PE is data-starved then anyway)
                # to start the DVE stt stream as early as possible
                def after_y2(ar, amp):
                    if amp == 1:
                        emit_l10(ar, range(0, 4))
                    if amp == 3:
                        emit_l10(ar, range(4, 8))
                        pend11.extend((ar, m) for m in range(8))

                if idx < 2:
                    emit_l9_and_y(r, mp)
                    after_y2(r, mp)
                else:
                    if prev_job is not None:
                        pr, pmp = prev_job
                        emit_l9_and_y(pr, pmp)
                        after_y2(pr, pmp)
                    prev_job = (r, mp)

                # ---- drain deferred L11 steps (DVE) + their stores
                for _ in range(3 if idx >= 10 else 2):
                    if pend11:
                        p_r, p_m = pend11.pop(0)
                        emit_l11(p_r, p_m)

            # tail: last job's L9/Y (+ its L10/L11 trigger), remaining L11
            pr, pmp = prev_job
            emit_l9_and_y(pr, pmp)
            if pmp == 1:
                emit_l10(pr, range(0, 4))
            if pmp == 3:
                emit_l10(pr, range(4, 8))
                pend11.extend((pr, m) for m in range(8))
            for p_r, p_m in pend11:
                emit_l11(p_r, p_m)

    nc.compile()
    return nc


def kernel(x, values, idx_in, idx_out):
    global LAST_EXEC_NS
    from concourse.bass_utils import run_bass_kernel_spmd

    x = np.asarray(x, np.float32)
    assert x.shape == (BATCH, N), x.shape
    mst, mstd, stabf = _host_precompute(values, idx_in, idx_out)
    xT = np.ascontiguousarray(x.T.astype(ml_dtypes.bfloat16))

    if "prog" not in _PROGRAM_CACHE:
        _PROGRAM_CACHE["prog"] = _build_program()
    nc = _PROGRAM_CACHE["prog"]

    in_maps = []
    for i in range(NCORES):
        xc = xT[:, i * BSH:(i + 1) * BSH]                  # [N, 512]
        # row m*128+p holds (lt, b) contiguously = H0 tile SBUF layout
        xs = np.ascontiguousarray(
            xc.reshape(8, 4, 128, BSH).transpose(0, 2, 1, 3).reshape(1024, 2048)
        )
        in_maps.append({"xT": xs, "mst": mst, "mstd": mstd, "stabf": stabf})
    res = run_bass_kernel_spmd(nc, in_maps, core_ids=list(range(NCORES)))
    if os.environ.get("BENES_TRACE"):
        tres = run_bass_kernel_spmd(
            nc, in_maps, core_ids=list(range(NCORES)), trace=True
        )
        LAST_EXEC_NS = tres.exec_time_ns
        _PROGRAM_CACHE["profile_json"] = tres.profile_json
    out = np.empty((BATCH, N), np.float32)
    for i in range(NCORES):
        out[i * BSH:(i + 1) * BSH] = (
            np.asarray(res.results[i]["outT"]).T.astype(np.float32)
        )
    return out
